# revision 2
# baseline (speedup 1.0000x reference)
"""GNN message-passing kernel for Trainium2 (8 NeuronCores).

Strategy: sort edges by tail node on host, shard tail-segments across the 8
cores (12500 segments each).  Each core processes its edges in 128-segment
"chunks"; edges of a chunk are padded to a uniform S subtiles of 128 edges.
All rel-table transforms are folded on host into small gatherable tables.
Per-edge gathers use GPSIMD indirect DMA (bf16); per-edge matmuls run on PE
in bf16; segment aggregation is a one-hot matmul into PSUM with exp(logit)
folded into the one-hot weights, so no DRAM scatter and no collectives.
"""

import os
import sys

import numpy as np

sys.path.insert(0, "/opt/trn_rl_repo")

import ml_dtypes  # noqa: E402

import concourse.bass as bass  # noqa: E402
import concourse.bacc as bacc  # noqa: E402
import concourse.mybir as mybir  # noqa: E402
from concourse.bass_utils import run_bass_kernel_spmd  # noqa: E402
from concourse.tile import TileContext  # noqa: E402

BF16 = mybir.dt.bfloat16
F32 = mybir.dt.float32
I32 = mybir.dt.int32
AF = mybir.ActivationFunctionType
OP = mybir.AluOpType

P = 128
H = 128
D = 100
N_CORES = 8
N_SEG = 100_000
SEG_PER_CORE = N_SEG // N_CORES  # 12500
CHUNKS = (SEG_PER_CORE + P - 1) // P  # 98 chunks of 128 segments
EPS = 1e-6
LN_EPS = 1e-5
NEG = -1.0e5  # added to dummy-edge logits -> exp == 0 in fp32

# knobs
GG = int(os.environ.get("KRN_GG", "2"))  # chunks per gather group
N_CHUNKS = int(os.environ.get("KRN_NCHUNKS", str(CHUNKS)))
TRACE = bool(int(os.environ.get("KRN_TRACE", "0")))
NO_GATHER = bool(int(os.environ.get("KRN_NO_GATHER", "0")))
NO_EPI = bool(int(os.environ.get("KRN_NO_EPI", "0")))
NO_MM = bool(int(os.environ.get("KRN_NO_MM", "0")))
REPEAT = int(os.environ.get("KRN_REPEAT", "1"))


def _bf(x):
    return np.ascontiguousarray(x.astype(ml_dtypes.bfloat16))


def _f32(x):
    return np.ascontiguousarray(x.astype(np.float32))


def _prep(inputs):
    """Host-side preprocessing: sorting, padding, table folding."""
    head = np.asarray(inputs["head_idx"]).astype(np.int32)
    rel = np.asarray(inputs["rel_idx"]).astype(np.int32)
    ent = np.asarray(inputs["ent_idx"]).astype(np.int32)
    tail = np.asarray(inputs["tail_idx"]).astype(np.int32)
    q = np.asarray(inputs["q_idx"]).astype(np.int32)
    node = _f32(np.asarray(inputs["node_emb"]))
    ent_t = _f32(np.asarray(inputs["ent_table"]))
    rel_t = _f32(np.asarray(inputs["rel_table"]))
    Ws = _f32(np.asarray(inputs["Ws"]))
    Wr = _f32(np.asarray(inputs["Wr"]))
    Wqr = _f32(np.asarray(inputs["Wqr"]))
    b_qr = _f32(np.asarray(inputs["b_qr"]))
    Wa = _f32(np.asarray(inputs["Wa"]))
    b_a = _f32(np.asarray(inputs["b_a"]))
    W_ih = _f32(np.asarray(inputs["W_ih"]))
    W_hh = _f32(np.asarray(inputs["W_hh"]))
    b_ih = _f32(np.asarray(inputs["b_ih"]))
    b_hh = _f32(np.asarray(inputs["b_hh"]))
    Wh = _f32(np.asarray(inputs["Wh"]))
    ln_g = _f32(np.asarray(inputs["ln_g"]))
    ln_b = _f32(np.asarray(inputs["ln_b"]))

    E = head.shape[0]

    # ---- sort edges by tail, bucket into cores and 128-seg chunks ----
    order = np.argsort(tail, kind="stable")
    t_s = tail[order]
    core_of = t_s // SEG_PER_CORE
    # chunk occupancy over all (core, chunk)
    gchunk = t_s // P  # global chunk id 0..CHUNKS*N_CORES-1 (since SEG_PER_CORE % P != 0 this is wrong)
    # careful: chunks are defined per-core on local tail ids
    lt_s = t_s - core_of * SEG_PER_CORE
    lchunk = lt_s // P

    n_gchunks = N_CORES * CHUNKS
    flat_chunk = core_of * CHUNKS + lchunk
    counts = np.bincount(flat_chunk, minlength=n_gchunks)
    S = int(max(1, int(np.ceil(counts.max() / P))))

    cap = S * P
    # position of each edge within its chunk (edges are sorted so chunks are contiguous runs)
    chunk_starts = np.zeros(n_gchunks + 1, np.int64)
    np.cumsum(counts, out=chunk_starts[1:])
    pos_in_chunk = np.arange(E, dtype=np.int64) - chunk_starts[flat_chunk]
    slot = flat_chunk * cap + pos_in_chunk  # destination slot in padded stream

    tot = n_gchunks * cap
    h_a = np.zeros(tot, np.int32)
    e_a = np.zeros(tot, np.int32)
    r_a = np.zeros(tot, np.int32)
    q_a = np.zeros(tot, np.int32)
    tr_a = np.full(tot, -1.0, np.float32)  # tail_rel, -1 for dummy (cast bf16 later)
    eb_a = np.full(tot, float(b_a[0]) + NEG, np.float32)

    h_a[slot] = head[order]
    e_a[slot] = ent[order]
    r_a[slot] = rel[order]
    q_a[slot] = q[order]
    tr_a[slot] = (lt_s - lchunk * P).astype(np.float32)
    eb_a[slot] = float(b_a[0])

    # reshape per core to [CHUNKS*S*P] then swizzle to [128, CHUNKS*S]
    def _sw(a):
        a = a.reshape(N_CORES, CHUNKS * S, P)
        return np.ascontiguousarray(np.transpose(a, (0, 2, 1)))  # [cores, 128, T]

    h_a, e_a, r_a, q_a, tr_a, eb_a = map(_sw, (h_a, e_a, r_a, q_a, tr_a, eb_a))

    # ---- folded tables ----
    A_rel = rel_t @ Wr.T  # [500, H]
    A_q = rel_t @ Wqr.T + b_qr  # [500, H]
    b_fold = b_ih + np.concatenate([b_hh[: 2 * H], np.zeros(H, np.float32)])
    G_rel = rel_t @ W_ih[:, D:].T + b_fold  # [500, 3H]
    G2 = np.concatenate([A_rel, G_rel], axis=1)  # [500, 512]

    ent_pad = np.zeros((ent_t.shape[0], P), np.float32)
    ent_pad[:, :D] = ent_t

    Wih_e = np.zeros((P, 3 * H), np.float32)
    Wih_e[:D, :] = W_ih[:, :D].T  # [128(K), 384]

    shared = {
        "node_bf": _bf(node),
        "ent_bf": _bf(ent_pad),
        "G2_bf": _bf(G2),
        "Aq_bf": _bf(A_q),
        "Ws_w": _bf(Ws.T),
        "Whh_rz": _bf(W_hh.T[:, : 2 * H]),
        "Whh_n": _bf(W_hh.T[:, 2 * H :]),
        "Wih_e": _bf(Wih_e),
        "Wh_w": _bf(Wh.T),
        "Wa_mat": _bf(np.tile(Wa[0], (P, 1))),
        "iota_mat": _bf(np.tile(np.arange(P, dtype=np.float32), (P, 1))),
        "idnt": _bf(np.eye(P, dtype=np.float32)),
        "ones1": _bf(np.ones((1, P), np.float32)),
        "bhhn_row": _bf(b_hh[2 * H :].reshape(1, H)),
        "ones_col": _bf(np.ones((P, 1), np.float32)),
        "lng_mat": _f32(np.tile(ln_g, (P, 1))),
        "lnb_mat": _f32(np.tile(ln_b, (P, 1))),
    }
    percore = []
    for c in range(N_CORES):
        percore.append(
            {
                "hidx": h_a[c],
                "eidx": e_a[c],
                "ridx": r_a[c],
                "qidx": q_a[c],
                "trel": tr_a[c],
                "ebias": eb_a[c],
            }
        )
    return shared, percore, S


def _build(S, n_chunks):
    """Build the Bass program (same for all cores)."""
    nc = bacc.Bacc("TRN2", debug=False)

    T = CHUNKS * S  # subtiles per core in the input arrays

    # DRAM tensors
    d_node = nc.dram_tensor("node_bf", [N_SEG, P], BF16, kind="ExternalInput")
    d_ent = nc.dram_tensor("ent_bf", [N_SEG, P], BF16, kind="ExternalInput")
    d_g2 = nc.dram_tensor("G2_bf", [500, 4 * H], BF16, kind="ExternalInput")
    d_aq = nc.dram_tensor("Aq_bf", [500, H], BF16, kind="ExternalInput")
    d_ws = nc.dram_tensor("Ws_w", [P, H], BF16, kind="ExternalInput")
    d_whhrz = nc.dram_tensor("Whh_rz", [P, 2 * H], BF16, kind="ExternalInput")
    d_whhn = nc.dram_tensor("Whh_n", [P, H], BF16, kind="ExternalInput")
    d_wihe = nc.dram_tensor("Wih_e", [P, 3 * H], BF16, kind="ExternalInput")
    d_wh = nc.dram_tensor("Wh_w", [P, H], BF16, kind="ExternalInput")
    d_wa = nc.dram_tensor("Wa_mat", [P, H], BF16, kind="ExternalInput")
    d_iota = nc.dram_tensor("iota_mat", [P, P], BF16, kind="ExternalInput")
    d_idnt = nc.dram_tensor("idnt", [P, P], BF16, kind="ExternalInput")
    d_ones1 = nc.dram_tensor("ones1", [1, P], BF16, kind="ExternalInput")
    d_bhhn = nc.dram_tensor("bhhn_row", [1, H], BF16, kind="ExternalInput")
    d_onesc = nc.dram_tensor("ones_col", [P, 1], BF16, kind="ExternalInput")
    d_lng = nc.dram_tensor("lng_mat", [P, H], F32, kind="ExternalInput")
    d_lnb = nc.dram_tensor("lnb_mat", [P, H], F32, kind="ExternalInput")

    d_hidx = nc.dram_tensor("hidx", [P, T], I32, kind="ExternalInput")
    d_eidx = nc.dram_tensor("eidx", [P, T], I32, kind="ExternalInput")
    d_ridx = nc.dram_tensor("ridx", [P, T], I32, kind="ExternalInput")
    d_qidx = nc.dram_tensor("qidx", [P, T], I32, kind="ExternalInput")
    d_trel = nc.dram_tensor("trel", [P, T], F32, kind="ExternalInput")
    d_ebias = nc.dram_tensor("ebias", [P, T], F32, kind="ExternalInput")

    d_out = nc.dram_tensor("out", [CHUNKS * P, H], F32, kind="ExternalOutput")

    W = GG * S  # subtiles per gather group

    with TileContext(nc) as tc:
        with (
            tc.tile_pool(name="const", bufs=1) as cp,
            tc.tile_pool(name="gather", bufs=6) as gp,
            tc.tile_pool(name="trs", bufs=4) as tp,
            tc.tile_pool(name="work", bufs=4) as wp,
            tc.tile_pool(name="ep", bufs=4) as ep,
            tc.tile_pool(name="ps_pre", bufs=2, space="PSUM") as pp_pre,
            tc.tile_pool(name="ps_g", bufs=2, space="PSUM") as pp_g,
            tc.tile_pool(name="ps_seg", bufs=2, space="PSUM") as pp_seg,
            tc.tile_pool(name="ps_tr", bufs=2, space="PSUM") as pp_tr,
        ):
            # ---- resident constants ----
            ws_w = cp.tile_from(d_ws[:])
            whh_rz = cp.tile_from(d_whhrz[:])
            whh_n = cp.tile_from(d_whhn[:])
            wih_e = cp.tile_from(d_wihe[:])
            wh_w = cp.tile_from(d_wh[:])
            wa_mat = cp.tile_from(d_wa[:])
            iota = cp.tile_from(d_iota[:])
            idnt = cp.tile_from(d_idnt[:])
            ones1 = cp.tile_from(d_ones1[:])
            bhhn = cp.tile_from(d_bhhn[:])
            onesc = cp.tile_from(d_onesc[:])
            lng = cp.tile_from(d_lng[:])
            lnb = cp.tile_from(d_lnb[:])
            hidx = cp.tile_from(d_hidx[:])
            eidx = cp.tile_from(d_eidx[:])
            ridx = cp.tile_from(d_ridx[:])
            qidx = cp.tile_from(d_qidx[:])
            trel = cp.tile_from(d_trel[:])
            ebias = cp.tile_from(d_ebias[:])

            seg_st = cp.tile([P, n_chunks, H + 4], F32)
            import contextlib
            rep_ctx = tc.For_i(0, REPEAT, 1) if REPEAT > 1 else contextlib.nullcontext()
            with rep_ctx:
                for chunk in range(n_chunks):
                    p_seg = pp_seg.tile([P, H + 1], F32, tag="seg")
                    for k in range(S):
                        st = chunk * S + k  # subtile within core stream
                        hs_t = gp.tile([P, H], BF16, tag="hs_t")
                        he_t = gp.tile([P, H], BF16, tag="he_t")
                        g2_t = gp.tile([P, 4 * H], BF16, tag="g2_t")
                        aq_t = gp.tile([P, H], BF16, tag="aq_t")
                        if not NO_GATHER:
                            nc.gpsimd.indirect_dma_start(
                                out=hs_t[:], out_offset=None, in_=d_node[:],
                                in_offset=bass.IndirectOffsetOnAxis(
                                    ap=hidx[:, st : st + 1], axis=0))
                            nc.gpsimd.indirect_dma_start(
                                out=he_t[:], out_offset=None, in_=d_ent[:],
                                in_offset=bass.IndirectOffsetOnAxis(
                                    ap=eidx[:, st : st + 1], axis=0))
                            nc.gpsimd.indirect_dma_start(
                                out=g2_t[:], out_offset=None, in_=d_g2[:],
                                in_offset=bass.IndirectOffsetOnAxis(
                                    ap=ridx[:, st : st + 1], axis=0))
                            nc.gpsimd.indirect_dma_start(
                                out=aq_t[:], out_offset=None, in_=d_aq[:],
                                in_offset=bass.IndirectOffsetOnAxis(
                                    ap=qidx[:, st : st + 1], axis=0))
                        else:
                            nc.sync.dma_start(hs_t[:], d_node[0:P, :])
                            nc.sync.dma_start(he_t[:], d_ent[0:P, :])
                            nc.sync.dma_start(g2_t[:], d_g2[0:P, :])
                            nc.sync.dma_start(aq_t[:], d_aq[0:P, :])
                        hs_sl = hs_t[:]
                        he_sl = he_t[:]

                        # transposes via PE (identity matmul) -> PSUM -> SBUF
                        p_tr = pp_tr.tile([P, 2, H], BF16, tag="tr")
                        nc.tensor.transpose(p_tr[:, 0, :], hs_sl, idnt[:])
                        nc.tensor.transpose(p_tr[:, 1, :], he_sl, idnt[:])
                        hheT = tp.tile([P, 2, H], BF16, tag="hheT")
                        nc.scalar.activation(hheT[:], p_tr[:], AF.Copy)
                        hsT = hheT[:, 0, :]
                        heT = hheT[:, 1, :]

                        # ---- attention pre ----
                        p_pre = pp_pre.tile([P, H], F32, tag="pre")
                        nc.tensor.matmul(
                            p_pre[:], idnt[:], g2_t[:, 0:H], start=True, stop=False
                        )
                        nc.tensor.matmul(
                            p_pre[:], idnt[:], aq_t[:], start=False, stop=False
                        )
                        nc.tensor.matmul(
                            p_pre[:], hsT, ws_w[:], start=False, stop=True
                        )
                        pre = wp.tile([P, H], BF16, tag="pre_s")
                        nc.scalar.activation(pre[:], p_pre[:], AF.Relu)

                        # logit = sum_f pre*Wa  (accum_out)
                        junk = wp.tile([P, H], BF16, tag="junk")
                        logit = wp.tile([P, 1], F32, tag="logit")
                        nc.vector.scalar_tensor_tensor(
                            out=junk[:],
                            in0=pre[:],
                            scalar=1.0,
                            in1=wa_mat[:],
                            op0=OP.mult,
                            op1=OP.mult,
                            accum_out=logit[:],
                        )
                        ex = wp.tile([P, 1], F32, tag="ex")
                        nc.scalar.activation(
                            ex[:], logit[:], AF.Exp, bias=ebias[:, st : st + 1]
                        )

                        # ---- GRU gates (one PSUM bank: [rz | xn | hn]) ----
                        p_g = pp_g.tile([P, 4 * H], F32, tag="g")
                        nc.tensor.matmul(
                            p_g[:, 0 : 3 * H], idnt[:], g2_t[:, H : 4 * H],
                            start=True, stop=False, skip_group_check=True,
                        )
                        nc.tensor.matmul(
                            p_g[:, 0 : 3 * H], heT, wih_e[:], start=False,
                            stop=False, skip_group_check=True,
                        )
                        nc.tensor.matmul(
                            p_g[:, 0 : 2 * H], hsT, whh_rz[:], start=False,
                            stop=False, skip_group_check=True,
                        )
                        nc.tensor.matmul(
                            p_g[:, 3 * H : 4 * H], ones1[:], bhhn[:], start=True,
                            stop=False, skip_group_check=True,
                        )
                        nc.tensor.matmul(
                            p_g[:, 3 * H : 4 * H], hsT, whh_n[:], start=False,
                            stop=True, skip_group_check=True,
                        )

                        rz = wp.tile([P, 2 * H], BF16, tag="rz")
                        nc.scalar.activation(rz[:], p_g[:, 0 : 2 * H], AF.Sigmoid)
                        xnhn = wp.tile([P, 2 * H], BF16, tag="xnhn")
                        nc.scalar.activation(xnhn[:], p_g[:, 2 * H : 4 * H], AF.Copy)
                        xn_s = xnhn[:, 0:H]
                        hn_s = xnhn[:, H : 2 * H]

                        t_t = wp.tile([P, H], BF16, tag="t_t")
                        nc.vector.tensor_mul(t_t[:], rz[:, 0:H], hn_s)
                        ni = wp.tile([P, H], BF16, tag="ni")
                        nc.vector.tensor_add(ni[:], xn_s, t_t[:])
                        n_t = wp.tile([P, H], BF16, tag="n_t")
                        nc.scalar.activation(n_t[:], ni[:], AF.Tanh)

                        d_t = wp.tile([P, H], BF16, tag="d_t")
                        nc.vector.tensor_sub(d_t[:], hs_sl, n_t[:])
                        zd = wp.tile([P, H], BF16, tag="zd")
                        nc.vector.tensor_mul(zd[:], rz[:, H : 2 * H], d_t[:])
                        rhs_t = wp.tile([P, H + 1], BF16, tag="rhs_t")
                        nc.vector.tensor_add(rhs_t[:, 0:H], n_t[:], zd[:])
                        nc.vector.tensor_copy(rhs_t[:, H : H + 1], onesc[:])

                        # one-hot with exp(logit) folded in
                        ohw = wp.tile([P, P], BF16, tag="ohw")
                        nc.vector.tensor_scalar(
                            out=ohw[:],
                            in0=iota[:],
                            scalar1=trel[:, st : st + 1],
                            scalar2=ex[:],
                            op0=OP.is_equal,
                            op1=OP.mult,
                        )
                        nc.tensor.matmul(
                            p_seg[:],
                            ohw[:],
                            rhs_t[:],
                            start=(k == 0),
                            stop=(k == S - 1),
                            skip_group_check=True,
                        )

                    st_c = seg_st[:, chunk, 0 : H + 1]
                    nc.scalar.activation(st_c, p_seg[:], AF.Copy)
                    if NO_EPI:
                        ob0 = ep.tile([P, H], F32, tag="ob")
                        nc.scalar.activation(ob0[:], p_seg[:, 0:H], AF.Copy)
                        nc.sync.dma_start(d_out[chunk * P : (chunk + 1) * P, :], ob0[:])

                if not NO_EPI:
                    for chunk in range(n_chunks):
                        # ---- chunk epilogue ----
                        de = ep.tile([P, 1], F32, tag="de")
                        nc.vector.tensor_scalar_add(de[:], seg_st[:, chunk, H : H + 1], EPS)
                        rd = ep.tile([P, 1], F32, tag="rd")
                        nc.vector.reciprocal(rd[:], de[:])
                        agg = ep.tile([P, H], BF16, tag="agg")
                        nc.vector.tensor_scalar_mul(agg[:], seg_st[:, chunk, 0:H], rd[:])
                        p_trE = pp_tr.tile([P, 2, H], BF16, tag="tr")
                        nc.tensor.transpose(p_trE[:, 0, :], agg[:], idnt[:])
                        aggT = ep.tile([P, H], BF16, tag="aggT")
                        nc.vector.tensor_copy(aggT[:], p_trE[:, 0, :])
                        p_o = pp_pre.tile([P, H], F32, tag="pre")
                        nc.tensor.matmul(p_o[:], aggT[:], wh_w[:], start=True, stop=True)
                        o_t = ep.tile([P, H], F32, tag="o_t")
                        s1 = ep.tile([P, 1], F32, tag="s1")
                        nc.scalar.activation(o_t[:], p_o[:], AF.Relu, accum_out=s1[:])
                        osq = ep.tile([P, H], F32, tag="osq")
                        s2 = ep.tile([P, 1], F32, tag="s2")
                        nc.scalar.activation(osq[:], o_t[:], AF.Square, accum_out=s2[:])
                        mu = ep.tile([P, 1], F32, tag="mu")
                        nc.vector.tensor_scalar_mul(mu[:], s1[:], 1.0 / H)
                        m2 = ep.tile([P, 1], F32, tag="m2")
                        nc.vector.tensor_scalar_mul(m2[:], s2[:], 1.0 / H)
                        mu2 = ep.tile([P, 1], F32, tag="mu2")
                        nc.vector.tensor_mul(mu2[:], mu[:], mu[:])
                        var = ep.tile([P, 1], F32, tag="var")
                        nc.vector.tensor_sub(var[:], m2[:], mu2[:])
                        nc.vector.tensor_scalar_add(var[:], var[:], LN_EPS)
                        sd = ep.tile([P, 1], F32, tag="sd")
                        nc.scalar.activation(sd[:], var[:], AF.Sqrt)
                        rstd = ep.tile([P, 1], F32, tag="rstd")
                        nc.vector.reciprocal(rstd[:], sd[:])
                        oc = ep.tile([P, H], F32, tag="oc")
                        nc.vector.tensor_scalar(
                            out=oc[:],
                            in0=o_t[:],
                            scalar1=mu[:],
                            scalar2=rstd[:],
                            op0=OP.subtract,
                            op1=OP.mult,
                        )
                        og = ep.tile([P, H], F32, tag="og")
                        nc.vector.tensor_mul(og[:], oc[:], lng[:])
                        ob = ep.tile([P, H], F32, tag="ob")
                        nc.vector.tensor_add(ob[:], og[:], lnb[:])
                        nc.sync.dma_start(
                            d_out[chunk * P : (chunk + 1) * P, :], ob[:]
                        )
    nc.finalize()
    return nc


def kernel(**inputs):
    shared, percore, S = _prep(inputs)
    nc = _build(S, N_CHUNKS)
    in_maps = []
    for c in range(N_CORES):
        m = dict(shared)
        m.update(percore[c])
        in_maps.append(m)
    tmpdir = os.environ.get("KRN_TMPDIR") or None
    if tmpdir:
        os.makedirs(tmpdir, exist_ok=True)
    res = run_bass_kernel_spmd(
        nc, in_maps, core_ids=list(range(N_CORES)), trace=TRACE, tmpdir=tmpdir
    )
    outs = [res.results[c]["out"][:SEG_PER_CORE] for c in range(N_CORES)]
    full = np.concatenate(outs, axis=0).astype(np.float32)
    kernel._last_exec_ns = res.exec_time_ns
    return full


if __name__ == "__main__":
    pass

